# revision 1
# baseline (speedup 1.0000x reference)
"""Trainium2 Bass kernel for cross-attention (efficient/linear attention variant).

Computation per batch b (fully batch-independent -> data parallel over 8 cores):
    q  = Wq @ x[b]                         # (128, N)
    kv = Wkv @ context[b].T                # (256, NCTX)
    k, v = kv[:128], kv[128:]
    q = softmax_d(q) * d**-0.5             # softmax over feature dim within head
    k = softmax_n(k)                       # softmax over sequence dim
    ctx[h] = k_h @ v_h.T                   # (32, 32) per head
    out[h] = ctx[h].T @ q_h                # (32, N)
    y = Wo @ out + bo

Strategy:
  - One batch per NeuronCore (8 cores), no collectives.
  - Host pre-transposes context to (512, NCTX) and casts streams to bf16 so no
    on-chip transposes are needed at all.
  - Phase A: stream ctxT, compute kvT = ctxT_chunk.T @ WkvT (n on partitions),
    exp(k) on ScalarE, then a single accumulating matmul builds
    C[(h,d),(h,e)] = sum_n exp(k) v and Z[(h,d)] = sum_n exp(k) (ones column).
  - Phase B: stream x, q-proj, exp, per-head sums via indicator matmul,
    reciprocal, broadcast via indicator matmul, block-diag ctx matmul,
    output projection (+bias) and DMA out.
"""

import os
import sys
from contextlib import ExitStack

import numpy as np

if "/opt/trn_rl_repo" not in sys.path:
    sys.path.insert(0, "/opt/trn_rl_repo")

import ml_dtypes

import concourse.bass as bass
from concourse import bacc
import concourse.mybir as mybir
import concourse.tile as tile
from concourse.bass_utils import run_bass_kernel_spmd

HEADS = 4
DIM_HEAD = 32
SCALE = DIM_HEAD**-0.5
B = 8
DIM = 256
N = 16384
NCTX = 16384
CDIM = 512
HID = HEADS * DIM_HEAD  # 128

BF16 = mybir.dt.bfloat16
F32 = mybir.dt.float32
EXP = mybir.ActivationFunctionType.Exp

TILE_N = 512  # free-dim tile for both phases
LN_OFF = 4.0  # constant offset on ln(S); e^-LN_OFF folded into Wo
LN_OFF_SCALE = float(np.exp(-LN_OFF))  # activation scale: ln(S*e^-off) = ln(S)-off


def build_graph(n: int = N, nctx: int = NCTX) -> bass.Bass:
    global N, NCTX
    saved = (N, NCTX)
    N, NCTX = n, nctx
    try:
        return _build_graph_impl()
    finally:
        N, NCTX = saved


def _build_graph_impl() -> bass.Bass:
    nc = bacc.Bacc()

    ctxt = nc.dram_tensor("ctxt", [CDIM, NCTX], BF16, kind="ExternalInput")
    xs = nc.dram_tensor("xs", [DIM, N], BF16, kind="ExternalInput")
    wqt = nc.dram_tensor("wqt", [DIM, HID], BF16, kind="ExternalInput")
    wkvt = nc.dram_tensor("wkvt", [CDIM, 2 * HID], BF16, kind="ExternalInput")
    wot = nc.dram_tensor("wot", [HID, DIM], BF16, kind="ExternalInput")
    bob = nc.dram_tensor("bob", [DIM], F32, kind="ExternalInput")
    ind4 = nc.dram_tensor("ind4", [HID, HEADS], BF16, kind="ExternalInput")
    ind128 = nc.dram_tensor("ind128", [HEADS, HID], BF16, kind="ExternalInput")
    bmask = nc.dram_tensor("bmask", [HID, HID], F32, kind="ExternalInput")
    y = nc.dram_tensor("y", [DIM, N], BF16, kind="ExternalOutput")

    with tile.TileContext(nc) as tc, ExitStack() as ctx:
        cpool = ctx.enter_context(tc.tile_pool(name="consts", bufs=1))

        wqt_sb = cpool.tile([128, 2, HID], BF16)
        nc.sync.dma_start(wqt_sb, wqt.rearrange("(cc p) m -> p cc m", p=128))
        wkvt_sb = cpool.tile([128, 4, 2 * HID], BF16)
        nc.sync.dma_start(wkvt_sb, wkvt.rearrange("(cc p) m -> p cc m", p=128))
        wot_sb = cpool.tile([128, 2, 128], BF16)
        nc.sync.dma_start(wot_sb, wot.rearrange("p (oc m) -> p oc m", oc=2))
        bo_sb = cpool.tile([128, 2], F32)
        nc.sync.dma_start(bo_sb, bob.rearrange("(oc p) -> p oc", p=128))
        ind4_sb = cpool.tile([HID, HEADS], BF16)
        nc.sync.dma_start(ind4_sb, ind4[:, :])
        ind128_sb = cpool.tile([HEADS, HID], BF16)
        nc.sync.dma_start(ind128_sb, ind128[:, :])
        bmask_sb = cpool.tile([HID, HID], F32)
        nc.sync.dma_start(bmask_sb, bmask[:, :])

        bd = cpool.tile([128, 128], BF16)  # block-diag ctx matrix (phase A out)
        ones_sb = cpool.tile([128, 1], BF16)
        nc.gpsimd.memset(ones_sb, 1.0)

        ctxt_r = ctxt.rearrange("(cc p) n -> p cc n", p=128)  # (128, 4, NCTX)
        xr = xs.rearrange("(cc p) n -> p cc n", p=128)  # (128, 2, N)
        yr = y.rearrange("(oc p) n -> p oc n", p=128)
        n_tiles = NCTX // TILE_N
        chunks = TILE_N // 128
        total_chunks = NCTX // 128

        # persistent intermediates bridging the phases
        eq_all = cpool.tile([128, NCTX // TILE_N, TILE_N], BF16)  # exp(q)
        s_all = cpool.tile([HEADS, N], F32)  # per-head column sums of exp(q)
        lns_all = cpool.tile([HEADS, N], BF16)  # ln(S) - LN_OFF

        # ------- Phase A: context -> C, Z ----------------------------------
        with (
            tc.tile_pool(name="actx", bufs=3) as apool,
            tc.tile_pool(name="akv", bufs=4) as kpool,
            tc.tile_pool(name="apsum", bufs=2, space="PSUM") as apsum,
            tc.tile_pool(name="czpsum", bufs=1, space="PSUM") as czpool,
        ):
            cz_ps = czpool.tile([128, HID], F32)
            z_ps = czpool.tile([128, 1], F32)
            for t in range(n_tiles):
                sl = slice(t * TILE_N, (t + 1) * TILE_N)
                ct = apool.tile([128, 4, TILE_N], BF16, tag="ct")
                nc.sync.dma_start(ct, ctxt_r[:, :, sl])

                for g in range(chunks // 2):  # groups of 2 chunks
                    kvt_ps = apsum.tile([128, 2, TILE_N], F32, tag="kvt")
                    for j2 in range(2):
                        j = g * 2 + j2
                        for cc in range(4):
                            nc.tensor.matmul(
                                kvt_ps[:, j2, 0 : 2 * HID],
                                ct[:, cc, j * 128 : (j + 1) * 128],
                                wkvt_sb[:, cc, :],
                                start=(cc == 0),
                                stop=(cc == 3),
                            )
                    kt = kpool.tile([128, 2, 128], BF16, tag="kt")
                    nc.scalar.activation(kt, kvt_ps[:, :, 0:HID], EXP)
                    vt = kpool.tile([128, 2, 128], BF16, tag="vt")
                    nc.vector.tensor_copy(vt, kvt_ps[:, :, HID : 2 * HID])
                    for j2 in range(2):
                        ci = t * chunks + g * 2 + j2
                        nc.tensor.matmul(
                            cz_ps,
                            kt[:, j2, :],
                            vt[:, j2, :],
                            start=(ci == 0),
                            stop=(ci == total_chunks - 1),
                        )
                        nc.tensor.matmul(
                            z_ps,
                            kt[:, j2, :],
                            ones_sb,
                            start=(ci == 0),
                            stop=(ci == total_chunks - 1),
                        )

            # BD = (C / Z_row) masked to block-diagonal
            rz = kpool.tile([128, 1], F32, tag="rz")
            nc.vector.reciprocal(rz, z_ps)
            bd0 = kpool.tile([128, 128], F32, tag="bd0")
            nc.vector.tensor_scalar_mul(bd0, cz_ps, rz)
            nc.vector.tensor_mul(bd, bd0, bmask_sb)

        # ------- Phase B1: x -> exp(q), S ----------------------------------
        with (
            tc.tile_pool(name="bx1", bufs=3) as b1pool,
            tc.tile_pool(name="bp1", bufs=2, space="PSUM") as b1psum,
        ):
            for t in range(n_tiles):
                sl = slice(t * TILE_N, (t + 1) * TILE_N)
                xt = b1pool.tile([128, 2, TILE_N], BF16, tag="xt")
                nc.sync.dma_start(xt, xr[:, :, sl])

                q_ps = b1psum.tile([128, TILE_N], F32, tag="q")
                for cc in range(2):
                    nc.tensor.matmul(
                        q_ps,
                        wqt_sb[:, cc, :],
                        xt[:, cc, :],
                        start=(cc == 0),
                        stop=(cc == 1),
                    )
                nc.scalar.activation(eq_all[:, t, :], q_ps, EXP)

                s_ps = b1psum.tile([HEADS, TILE_N], F32, tag="s")
                nc.tensor.matmul(
                    s_ps, ind4_sb, eq_all[:, t, :], start=True, stop=True
                )
                nc.vector.tensor_copy(s_all[:, sl], s_ps)

            # One batched ln pass -> only one ACT LUT swap to Ln and back.
            # ln(S * e^-LN_OFF) = ln(S) - LN_OFF keeps the bf16 value near 0
            # (small abs error); e^-LN_OFF is folded into wot on the host.
            LN_CHUNK = N // 4
            for i in range(4):
                lsl = slice(i * LN_CHUNK, (i + 1) * LN_CHUNK)
                nc.scalar.activation(
                    lns_all[:, lsl],
                    s_all[:, lsl],
                    mybir.ActivationFunctionType.Ln,
                    scale=LN_OFF_SCALE,
                )

        # ------- Phase B2: broadcast 1/S, ctx matrix, output proj ----------
        with (
            tc.tile_pool(name="bx2", bufs=3) as b2pool,
            tc.tile_pool(name="be2", bufs=3) as e2pool,
            tc.tile_pool(name="bp2", bufs=2, space="PSUM") as b2psum,
        ):
            for t in range(n_tiles):
                sl = slice(t * TILE_N, (t + 1) * TILE_N)
                rb_ps = b2psum.tile([128, TILE_N], F32, tag="rb")
                nc.tensor.matmul(
                    rb_ps, ind128_sb, lns_all[:, sl], start=True, stop=True
                )
                rb = e2pool.tile([128, TILE_N], BF16, tag="rbs")
                nc.scalar.activation(rb, rb_ps, EXP, scale=-1.0)

                u_ps = b2psum.tile([128, TILE_N], F32, tag="u")
                nc.tensor.matmul(u_ps, bd, eq_all[:, t, :], start=True, stop=True)
                un = e2pool.tile([128, TILE_N], BF16, tag="un")
                nc.vector.tensor_mul(un, u_ps, rb)

                yt = b2pool.tile([128, 2, TILE_N], BF16, tag="yt")
                for oc in range(2):
                    y_ps = b2psum.tile([128, TILE_N], F32, tag="y")
                    nc.tensor.matmul(
                        y_ps, wot_sb[:, oc, :], un, start=True, stop=True
                    )
                    if oc == 0:
                        nc.scalar.add(yt[:, oc, :], y_ps, bo_sb[:, oc : oc + 1])
                    else:
                        nc.vector.tensor_scalar_add(
                            yt[:, oc, :], y_ps, bo_sb[:, oc : oc + 1]
                        )
                nc.sync.dma_start(yr[:, :, sl], yt)

    nc.compile()
    return nc


_GRAPH_CACHE: dict = {}


def _prep_inputs(x, context, Wq, Wkv, Wo, bo):
    bf16 = ml_dtypes.bfloat16
    x = np.asarray(x, dtype=np.float32)
    context = np.asarray(context, dtype=np.float32)
    Wq = np.asarray(Wq, dtype=np.float32)
    Wkv = np.asarray(Wkv, dtype=np.float32)
    Wo = np.asarray(Wo, dtype=np.float32)
    bo = np.asarray(bo, dtype=np.float32)

    wqt = np.ascontiguousarray(Wq.T).astype(bf16)              # (256, 128)
    wkvt = np.ascontiguousarray(Wkv.T).astype(bf16)            # (512, 256)
    wot = np.ascontiguousarray((Wo * SCALE * np.exp(-LN_OFF)).T).astype(bf16)  # (128, 256)

    ind4 = np.zeros((HID, HEADS), dtype=bf16)
    ind4[np.arange(HID), np.arange(HID) // DIM_HEAD] = 1
    ind128 = np.ascontiguousarray(ind4.T)
    bmask = (
        (np.arange(HID)[:, None] // DIM_HEAD) == (np.arange(HID)[None, :] // DIM_HEAD)
    ).astype(np.float32)

    in_maps = []
    for b in range(B):
        in_maps.append(
            {
                "ctxt": np.ascontiguousarray(context[b].T).astype(bf16),
                "xs": x[b].astype(bf16),
                "wqt": wqt,
                "wkvt": wkvt,
                "wot": wot,
                "bob": bo,
                "ind4": ind4,
                "ind128": ind128,
                "bmask": bmask,
            }
        )
    return in_maps


def run(inputs: dict, trace: bool = False):
    if "nc" not in _GRAPH_CACHE:
        _GRAPH_CACHE["nc"] = build_graph()
    nc = _GRAPH_CACHE["nc"]
    in_maps = _prep_inputs(**inputs)
    res = run_bass_kernel_spmd(nc, in_maps, core_ids=list(range(B)), trace=trace)
    out = np.stack(
        [np.asarray(res.results[b]["y"], dtype=np.float32) for b in range(B)]
    )
    return out, res


def kernel(**inputs) -> np.ndarray:
    out, _ = run(inputs, trace=False)
    return out

